# revision 55
# baseline (speedup 1.0000x reference)
"""DSS (Diagonal State Space) layer as a Bass/Tile kernel for 8 Trainium2 NeuronCores.

Algorithm (per core, channels H sharded 8 x 128):
  1. Build the DSS-exp kernel k[l,h] = Re(sum_n W[h,n] z[h,n]^l), z = exp(dt_h * Lambda_n),
     on-device via a two-level power factorization l = 32a + b:
       GW[h,n,b] = W * z^b (b<32),  Z32[h,n,a] = z^(32a) (a<16), both by complex doubling,
     then a per-channel PE matmul contracts the 64 modes (re/im packed into 128 partitions).
  2. K_f = rfft_1024(k) via PE matmuls against host-precomputed DFT tiles.
  3. Overlap-save FFT convolution: per 512-sample block, forward rfft-1024 as PE matmuls
     (packed 512-frequency layout, Nyquist folded into the sin-tile f=0 slot), complex
     pointwise multiply split across DVE/GPSIMD, inverse rfft as PE matmuls producing the
     valid 512 samples.
  4. The skip connection y += u * D is folded into the frequency-domain filter
     (K'_f = K_f + D), so it costs nothing in the main loop.

All matmuls use float32r (full PE rate; ~1.6e-4 relative rounding). Transcendentals are
evaluated with small-argument polynomials on DVE (the ACT LUTs are only ~1e-4 accurate,
which would compound through the z^511 power chains). The forward runs one block ahead
of the inverse in the PE stream; DMA traffic is spread across the SP/ACT HWDGE queues.
"""

import sys

for _p in ("/opt/trn_rl_repo", "/opt/trn_rl_repo/concourse"):
    if _p not in sys.path:
        sys.path.insert(0, _p)

import numpy as np
from contextlib import ExitStack

import concourse.bacc as bacc
import concourse.tile as tile
import concourse.mybir as mybir

dt = mybir.dt
f32 = np.float32

B, L, H, N = 4, 4096, 1024, 64
LK = 512
F = 1024          # FFT length (overlap-save)
HOP = 512         # block hop
NCORES = 8
HS = H // NCORES  # 128 channels per core
NBLK = L // HOP   # 8
NFT = 4           # packed frequency tiles (512 freqs + Nyquist folded)
NJ = F // 128     # 8 contraction chunks for the forward DFT
NLT = HOP // 128  # 4 output l-tiles per block
NCH = L // 128    # 32 u chunks per core


# ---------------------------------------------------------------- host constants
def build_constants():
    l = np.arange(F, dtype=np.float64)[:, None]
    f = np.arange(512, dtype=np.float64)[None, :]
    ang = 2 * np.pi * l * f / F
    C = np.cos(ang)
    S = -np.sin(ang)
    S[:, 0] = (-1.0) ** np.arange(F)      # Nyquist row packed into sin-tile col 0
    CF = np.zeros((NJ, 2, NFT, 128, 128))
    for j in range(NJ):
        for ft in range(NFT):
            CF[j, 0, ft] = C[128 * j:128 * j + 128, 128 * ft:128 * ft + 128]
            CF[j, 1, ft] = S[128 * j:128 * j + 128, 128 * ft:128 * ft + 128]
    lc = 512 + np.arange(512, dtype=np.float64)[None, :]   # valid circular outputs
    fr = np.arange(512, dtype=np.float64)[:, None]
    cf_ = np.where(fr == 0, 1.0, 2.0)
    Ar = cf_ * np.cos(2 * np.pi * fr * lc / F) / F
    Ai = -(2.0 / F) * np.sin(2 * np.pi * fr * lc / F)
    Ai[0, :] = ((-1.0) ** lc[0]) / F                        # Nyquist inverse row
    AI = np.zeros((2, NFT, NLT, 128, 128))
    for ft in range(NFT):
        for lt in range(NLT):
            AI[0, ft, lt] = Ar[128 * ft:128 * ft + 128, 128 * lt:128 * lt + 128]
            AI[1, ft, lt] = Ai[128 * ft:128 * ft + 128, 128 * lt:128 * lt + 128]
    return CF.astype(f32), AI.astype(f32)


# Horner coefficient lists (highest degree first)
def _fact(k):
    r = 1.0
    for i in range(2, k + 1):
        r *= i
    return r


EXP10 = [1.0 / _fact(k) for k in range(10, -1, -1)]          # e^x, |x| <~ 0.9
EXP9 = [1.0 / _fact(k) for k in range(9, -1, -1)]            # e^x, |x| <~ 0.4
SIN9 = [1.0 / _fact(9), -1.0 / _fact(7), 1.0 / _fact(5), -1.0 / _fact(3), 1.0]   # odd, in u = x^2
COSC = [1.0 / _fact(10), -1.0 / _fact(8), 1.0 / _fact(6), -1.0 / _fact(4),
        1.0 / _fact(2)]          # cos(x) = 1 - u*POLY? see _cos_poly


class _Prog:
    def __init__(self):
        self.nc = None
        self.built = False


_prog = _Prog()


def _emit_kernel(nc, tc, ctx, aps):
    V = nc.vector
    A = nc.scalar
    T = nc.tensor
    u_ap = aps["u"]; y_ap = aps["y"]
    cf_ap = aps["CF"]; ai_ap = aps["AI"]
    TT = V.tensor_tensor
    GT = nc.gpsimd.tensor_tensor
    op = mybir.AluOpType

    # ---------------- pools
    p_cf = ctx.enter_context(tc.tile_pool(name="cf", bufs=1))
    p_ai = ctx.enter_context(tc.tile_pool(name="ai", bufs=1))
    p_uch = ctx.enter_context(tc.tile_pool(name="uch", bufs=8))
    p_yf = ctx.enter_context(tc.tile_pool(name="yf", bufs=8))
    p_krep = ctx.enter_context(tc.tile_pool(name="krep", bufs=1))
    p_tmp = ctx.enter_context(tc.tile_pool(name="tmp", bufs=2))
    p_yout = ctx.enter_context(tc.tile_pool(name="yout", bufs=2))
    p_kc = ctx.enter_context(tc.tile_pool(name="kc", bufs=4))
    p_gw = ctx.enter_context(tc.tile_pool(name="gw", bufs=1))
    p_z32 = ctx.enter_context(tc.tile_pool(name="z32", bufs=1))
    p_zp = ctx.enter_context(tc.tile_pool(name="zp", bufs=9))
    p_small = ctx.enter_context(tc.tile_pool(name="small", bufs=1))
    p_gwtmp = ctx.enter_context(tc.tile_pool(name="gwtmp", bufs=1))
    p_drep = ctx.enter_context(tc.tile_pool(name="drep", bufs=1))
    p_ps = ctx.enter_context(tc.tile_pool(name="ps", bufs=4, space="PSUM"))
    p_psk = ctx.enter_context(tc.tile_pool(name="psk", bufs=2, space="PSUM"))

    def fview(t):
        return t[:].bitcast(dt.float32)

    # ---------------- small parameter tiles first (they gate the whole k prologue)
    logdt = p_small.tile([1, HS], dt.float32, tag="logdt")
    A.dma_start(logdt[:], aps["logdt"][:])
    Lre = p_small.tile([1, N], dt.float32, tag="lre")
    A.dma_start(Lre[:], aps["Lre"][:])
    Lim_r = p_small.tile([1, N], dt.float32, tag="lim")
    A.dma_start(Lim_r[:], aps["Lim"][:])

    # ---------------- constant stationaries, merged DMAs spread over both HWDGE queues
    cf_big = {}
    _cfq = [0]

    def load_cf(j, t_):
        tl = p_cf.tile([128, 4, 128], dt.float32r, tag=f"cf{j}_{t_}", name=f"cfb{j}_{t_}")
        eng = (nc.sync, nc.scalar)[_cfq[0] % 2]
        _cfq[0] += 1
        eng.dma_start(tl[:], cf_ap[j, t_].transpose([1, 0, 2]))
        cf_big[(j, t_)] = tl

    for j in (4, 5, 6, 7):
        for t_ in range(2):
            load_cf(j, t_)

    def cf_tile(j, t_, ft):
        return cf_big[(j, t_)][:, ft, :]


    # u chunks: prefetch the first window up front so the in-order DMA queues
    # don't head-of-line block them behind the k-prologue traffic
    chunks = {}

    def get_chunk(c):
        assert c >= 0
        if c not in chunks:
            t_u = p_uch.tile([128, 4, 128], dt.float32r, tag="uch", name=f"uch{c}")
            eng = nc.sync if c % 2 == 0 else nc.scalar
            eng.dma_start(t_u[:], u_ap[:, 128 * c:128 * c + 128, :].transpose([1, 0, 2]))
            chunks[c] = t_u
        return chunks[c]

    for c in range(8):
        get_chunk(c)

    for j in (0, 1, 2, 3):
        for t_ in range(2):
            load_cf(j, t_)

    # inverse stationaries load after the forward-critical traffic
    ai_big = {}
    for t_ in range(2):
        for ft in range(NFT):
            tl = p_ai.tile([128, 4, 128], dt.float32r, tag=f"ai{t_}_{ft}", name=f"aib{t_}_{ft}")
            eng = nc.sync if (ft + t_) % 2 == 0 else nc.scalar
            eng.dma_start(tl[:], ai_ap[t_, ft].transpose([1, 0, 2]))
            ai_big[(t_, ft)] = tl

    def ai_tile(t_, ft, lt):
        return ai_big[(t_, ft)][:, lt, :]

    def emit_fwd(blk):
        out = []
        for ft in range(NFT):
            pc = p_ps.tile([128, 512], dt.float32, tag="ps", name=f"pc{blk}_{ft}")
            psn = p_ps.tile([128, 512], dt.float32, tag="ps", name=f"psn{blk}_{ft}")
            first = True
            for j in range(NJ):
                c = 4 * blk - 4 + j
                if c < 0:
                    continue
                ch = get_chunk(c)
                T.matmul(pc[:], cf_tile(j, 0, ft), ch[:].rearrange("p b h -> p (b h)"),
                         start=first, stop=(j == NJ - 1))
                T.matmul(psn[:], cf_tile(j, 1, ft), ch[:].rearrange("p b h -> p (b h)"),
                         start=first, stop=(j == NJ - 1))
                first = False
            # evacuate psum on ACT so the DVE/GPS pointwise runs all-SBUF
            uc = p_tmp.tile([128, 512], dt.float32, tag="uc", bufs=7, name=f"uc{blk}_{ft}")
            A.copy(uc[:], pc[:])
            us = p_tmp.tile([128, 512], dt.float32, tag="us", bufs=7, name=f"us{blk}_{ft}")
            A.copy(us[:], psn[:])
            out.append((uc, us))
        return out

    fwd_done = {0: emit_fwd(0), 1: emit_fwd(1)}
    PREFETCH3 = True

    def horner_exp(dst, x, coefs):
        # dst = e^x via Horner in x; dst/x are [p, w] fp32 APs; uses p_small temps
        p = dst
        V.memset(p, float(coefs[0]))
        for c in coefs[1:]:
            tq = p_small.tile([x.shape[0], x.shape[1]], dt.float32, tag="horner", bufs=2)
            TT(tq[:], p, x, op.mult)
            V.tensor_scalar_add(p, tq[:], float(c))

    # dt = exp(logdt) = (exp(logdt/8))^8
    x8 = p_small.tile([1, HS], dt.float32, tag="x8")
    V.tensor_scalar_mul(x8[:], logdt[:], 0.125)
    e8 = p_small.tile([1, HS], dt.float32, tag="e8")
    horner_exp(e8[:], x8[:], EXP10)
    dtv = p_small.tile([1, HS], dt.float32, tag="dtv")
    t_a = p_small.tile([1, HS], dt.float32, tag="sq1")
    TT(t_a[:], e8[:], e8[:], op.mult)
    t_b = p_small.tile([1, HS], dt.float32, tag="sq2")
    TT(t_b[:], t_a[:], t_a[:], op.mult)
    TT(dtv[:], t_b[:], t_b[:], op.mult)

    # -exp(Lre) = -(exp(Lre/8))^8
    xl = p_small.tile([1, N], dt.float32, tag="xl")
    V.tensor_scalar_mul(xl[:], Lre[:], 0.125)
    el8 = p_small.tile([1, N], dt.float32, tag="el8")
    horner_exp(el8[:], xl[:], EXP10)
    t_c = p_small.tile([1, N], dt.float32, tag="sq3")
    TT(t_c[:], el8[:], el8[:], op.mult)
    t_d = p_small.tile([1, N], dt.float32, tag="sq4")
    TT(t_d[:], t_c[:], t_c[:], op.mult)
    negel = p_small.tile([1, N], dt.float32, tag="negel")
    t_e = p_small.tile([1, N], dt.float32, tag="sq5")
    TT(t_e[:], t_d[:], t_d[:], op.mult)
    V.tensor_scalar_mul(negel[:], t_e[:], -1.0)

    # outer products: a[n,h] = -e^{Lre_n} dt_h ; b[n,h] = Lim_n dt_h
    ps_a = p_psk.tile([128, 512], dt.float32, tag="psk")
    T.matmul(ps_a[0:N, 0:HS], negel[:], dtv[:], start=True, stop=True)
    ps_b = p_psk.tile([128, 512], dt.float32, tag="psk")
    T.matmul(ps_b[0:N, 0:HS], Lim_r[:], dtv[:], start=True, stop=True)

    # half-angle pieces on [N, HS]
    ah = p_small.tile([N, HS], dt.float32, tag="ah")
    V.tensor_scalar_mul(ah[:], ps_a[0:N, 0:HS], 0.5)
    bh = p_small.tile([N, HS], dt.float32, tag="bh")
    V.tensor_scalar_mul(bh[:], ps_b[0:N, 0:HS], 0.5)
    ea = p_small.tile([N, HS], dt.float32, tag="ea")
    horner_exp(ea[:], ah[:], EXP9)
    # sin(bh), cos(bh) via u = bh^2
    ub = p_small.tile([N, HS], dt.float32, tag="ub")
    TT(ub[:], bh[:], bh[:], op.mult)
    sp = p_small.tile([N, HS], dt.float32, tag="sp")
    V.memset(sp[:], float(SIN9[0]))
    for c in SIN9[1:]:
        tq = p_small.tile([N, HS], dt.float32, tag="horner", bufs=2)
        TT(tq[:], sp[:], ub[:], op.mult)
        V.tensor_scalar_add(sp[:], tq[:], float(c))
    sb = p_small.tile([N, HS], dt.float32, tag="sb")
    TT(sb[:], sp[:], bh[:], op.mult)          # sin(b/2)
    cp = p_small.tile([N, HS], dt.float32, tag="cp")
    V.memset(cp[:], float(COSC[0]))
    for c in COSC[1:]:
        tq = p_small.tile([N, HS], dt.float32, tag="horner", bufs=2)
        TT(tq[:], cp[:], ub[:], op.mult)
        V.tensor_scalar_add(cp[:], tq[:], float(c))
    # cos(x) = 1 - u * cp  (cp = 1/2 - u/24 + ... evaluated via Horner above)
    cb = p_small.tile([N, HS], dt.float32, tag="cb")
    tq = p_small.tile([N, HS], dt.float32, tag="horner", bufs=2)
    TT(tq[:], cp[:], ub[:], op.mult)
    V.tensor_scalar(cb[:], tq[:], -1.0, 1.0, op.mult, op.add)

    wre = p_small.tile([N, HS], dt.float32, tag="wre")
    TT(wre[:], ea[:], cb[:], op.mult)
    wim = p_small.tile([N, HS], dt.float32, tag="wim")
    TT(wim[:], ea[:], sb[:], op.mult)

    # complex squaring on separate re/im planes (all base-partition 0, lane-aligned)
    def csq_parts(dre, dim_, sre, sim):
        t1 = p_small.tile([N, HS], dt.float32, tag="csq1", bufs=2)
        TT(t1[:], sre, sre, op.mult)
        t2 = p_small.tile([N, HS], dt.float32, tag="csq2", bufs=2)
        TT(t2[:], sim, sim, op.mult)
        TT(dre, t1[:], t2[:], op.subtract)
        t3 = p_small.tile([N, HS], dt.float32, tag="csq3", bufs=2)
        TT(t3[:], sre, sim, op.mult)
        V.tensor_scalar_mul(dim_, t3[:], 2.0)

    def new_zpair(nm):
        zr = p_zp.tile([N, HS], dt.float32, tag="zp", name=f"{nm}r")
        zi = p_zp.tile([N, HS], dt.float32, tag="zp", name=f"{nm}i")
        return zr, zi


    # ---------------- GW planes [N, HS, 32] holding (Re, -Im) of W z^b
    GWre_r = p_gw.tile([N, HS, 32], dt.float32r, tag="gwre")
    GWim_r = p_gw.tile([N, HS, 32], dt.float32r, tag="gwim")   # stores -Im
    GWre = GWre_r[:]
    GWim = GWim_r[:]
    A.dma_start(GWre[:, :, 0], aps["Wre"][:].bitcast(dt.float32r))
    wimt = p_small.tile([N, HS], dt.float32, tag="wimt")
    A.dma_start(wimt[:], aps["Wim"][:])
    V.tensor_scalar_mul(GWim[:, :, 0], wimt[:], -1.0)

    def cdouble_seg(pre, pim, zr, zi, s0, d0, w, conj_stored):
        # planes [.., d0:d0+w] = planes[.., s0:s0+w] * (zr + i zi);
        # when conj_stored, the im plane holds the negated imaginary part.
        zre = zr[:].unsqueeze(2).broadcast_to([N, HS, w])
        zim = zi[:].unsqueeze(2).broadcast_to([N, HS, w])
        t2 = p_gwtmp.tile([N, HS, 8], dt.float32, tag="gt2", bufs=3)
        t4 = p_gwtmp.tile([N, HS, 8], dt.float32, tag="gt2", bufs=3)
        TT(pre[:, :, d0:d0 + w], pre[:, :, s0:s0 + w], zre, op.mult)
        GT(t2[:, :, 0:w], pim[:, :, s0:s0 + w], zim, op.mult)
        TT(pim[:, :, d0:d0 + w], pim[:, :, s0:s0 + w], zre, op.mult)
        GT(t4[:, :, 0:w], pre[:, :, s0:s0 + w], zim, op.mult)
        TT(pre[:, :, d0:d0 + w], pre[:, :, d0:d0 + w], t2[:, :, 0:w],
           op.add if conj_stored else op.subtract)
        TT(pim[:, :, d0:d0 + w], pim[:, :, d0:d0 + w], t4[:, :, 0:w],
           op.subtract if conj_stored else op.add)

    def cdouble(pre, pim, zr, zi, w, conj_stored):
        cdouble_seg(pre, pim, zr, zi, 0, w, w, conj_stored)


    # ---------------- Z32 planes [N, HS, 16] natural complex z^(32a)
    Zre_r = p_z32.tile([N, HS, 16], dt.float32r, tag="z32re")
    Zim_r = p_z32.tile([N, HS, 16], dt.float32r, tag="z32im")
    Zre = Zre_r[:]
    Zim = Zim_r[:]
    # a=0 plane is the complex constant 1+0i (memset can't emit float32r)
    V.tensor_scalar(Zre[:, :, 0], wre[:], 0.0, 1.0, op.mult, op.add)
    V.tensor_scalar(Zim[:, :, 0], wre[:], 0.0, 0.0, op.mult, op.add)

    # interleaved power chain + doubling: GW level j follows zp[j] immediately,
    # Z32 level j follows za[j], keeping the serial latency to a minimum
    zp = []
    z0 = new_zpair("z0")
    csq_parts(z0[0][:], z0[1][:], wre[:], wim[:])
    zp.append(z0)
    cdouble(GWre, GWim, zp[0][0], zp[0][1], 1, conj_stored=True)
    for j in range(1, 5):                     # z^2, z^4, z^8, z^16
        zj = new_zpair(f"z{1 << j}")
        csq_parts(zj[0][:], zj[1][:], zp[-1][0][:], zp[-1][1][:])
        zp.append(zj)
        if j < 4:
            cdouble(GWre, GWim, zp[j][0], zp[j][1], 1 << j, conj_stored=True)
    za = []
    z32t = new_zpair("z32")
    csq_parts(z32t[0][:], z32t[1][:], zp[4][0][:], zp[4][1][:])
    za.append(z32t)                           # z^32
    cdouble_seg(GWre, GWim, zp[4][0], zp[4][1], 0, 16, 8, conj_stored=True)
    cdouble_seg(GWre, GWim, zp[4][0], zp[4][1], 8, 24, 8, conj_stored=True)
    cdouble(Zre, Zim, za[0][0], za[0][1], 1, conj_stored=False)
    for j in range(1, 4):                     # z^64, z^128, z^256
        zj = new_zpair(f"za{j}")
        csq_parts(zj[0][:], zj[1][:], za[-1][0][:], za[-1][1][:])
        za.append(zj)
        cdouble(Zre, Zim, za[j][0], za[j][1], 1 << j, conj_stored=False)

    # ---------------- mode-sum: k[32a+b, h], two contraction-64 matmuls per channel
    # psum += GWre_h^T @ Zre_h ; psum += GWim_h^T @ Zim_h  (im plane is negated)
    ks = []
    for g in range(4):
        kp_g = p_psk.tile([32, 32, 16], dt.float32, tag="psk", name=f"kp{g}")
        for hl in range(32):
            h = 32 * g + hl
            T.matmul(kp_g[0:32, hl, :], GWre_r[:, h, :], Zre_r[:, h, :],
                     start=True, stop=False)
            T.matmul(kp_g[0:32, hl, :], GWim_r[:, h, :], Zim_r[:, h, :],
                     start=False, stop=True)
        # evacuate lane-aligned with a-major free order (contiguous shuffle reads)
        t_ks = p_yout.tile([32, 16, 32], dt.float32r, tag="yout", name=f"ks{g}")
        A.copy(t_ks[:], kp_g[:].transpose([0, 2, 1]))
        ks.append(t_ks)
    kc = []
    for c in range(4):
        kc.append(p_kc.tile([128, 128], dt.float32r, tag="kc", name=f"kc{c}"))
    kqi = 0
    for g in range(4):
        for c in range(4):
            for al in range(4):
                eng = (nc.sync, nc.scalar, nc.gpsimd)[kqi % 3]
                kqi += 1
                eng.dma_start(kc[c][:][32 * al:32 * al + 32, 32 * g:32 * g + 32],
                              ks[g][0:32, 4 * c + al, :])

    # ---------------- D_rep [128, 128] (D broadcast down partitions; folded into K)
    dtile = p_small.tile([1, HS], dt.float32, tag="dtile")
    A.dma_start(dtile[:], aps["D"][:])
    ones = p_small.tile([1, 128], dt.float32, tag="ones")
    V.memset(ones[:], 1.0)
    ps_d = p_psk.tile([128, 512], dt.float32, tag="psk")
    T.matmul(ps_d[0:128, 0:HS], ones[:], dtile[:], start=True, stop=True)
    D_rep = p_drep.tile([128, 128], dt.float32, tag="drep")
    A.copy(D_rep[:], ps_d[0:128, 0:HS])

    # ---------------- K_f via packed DFT (reuse forward stationaries j=0..3)
    kdft_ps = {}
    pks = {}
    for t_ in range(2):
        pks[t_] = p_psk.tile([128, 4, 128], dt.float32, tag="psk", name=f"kdft{t_}")
    for ft in range(NFT):
        for t_ in range(2):
            for c in range(4):
                T.matmul(pks[t_][:, ft, :], cf_tile(c, t_, ft), kc[c][:],
                         start=(c == 0), stop=(c == 3))
            kdft_ps[(t_, ft)] = pks[t_][:, ft, :]

    # Krep tensors [128, 128]; the pointwise broadcasts them across the 4 batch groups
    zrow = p_small.tile([1, 128], dt.float32, tag="zrow")
    V.memset(zrow[:], 0.0)
    # the skip connection u*D folds into the filter: K'_f = K_f + D (real part, all f)
    krA, krBC = [], []
    for ft in range(NFT):
        ta = p_krep.tile([128, 128], dt.float32r, tag=f"krA{ft}")
        tb = p_krep.tile([128, 128], dt.float32r, tag=f"krB{ft}")
        TT(ta[:], kdft_ps[(0, ft)], D_rep[:], op.add)
        A.copy(tb[:], kdft_ps[(1, ft)])
        krA.append(ta)
        krBC.append(tb)
    krD0 = p_krep.tile([128, 128], dt.float32r, tag="krD0")
    TT(krD0[:], kdft_ps[(0, 0)], D_rep[:], op.add)
    # row 0 of D-tensor holds K512r (from the packed sin psum row 0), plus D
    TT(krD0[0:1, :], kdft_ps[(1, 0)][0:1, :], D_rep[0:1, :], op.add)
    V.tensor_scalar(krBC[0][0:1, :], zrow[:], 0.0, 0.0, op.mult, op.add)     # Ki slot for f=0/Nyquist is zero

    # ---------------- D_rep [128, 512]
    dtile = p_small.tile([1, HS], dt.float32, tag="dtile")
    A.dma_start(dtile[:], aps["D"][:])
    ones = p_small.tile([1, 128], dt.float32, tag="ones")
    V.memset(ones[:], 1.0)
    ps_d = p_psk.tile([128, 512], dt.float32, tag="psk")
    T.matmul(ps_d[0:128, 0:HS], ones[:], dtile[:], start=True, stop=True)
    D_rep = p_drep.tile([128, 128], dt.float32, tag="drep")
    A.copy(D_rep[:], ps_d[0:128, 0:HS])

    # ---------------- main loop: overlap-save blocks

    def kb(t):
        return t[:].unsqueeze(1).broadcast_to([128, 4, 128])

    fwd_done[2] = emit_fwd(2)

    for blk in range(NBLK):
        yr_t, yi_t = [], []
        fwd = fwd_done.pop(blk)
        if blk + 1 < NBLK and blk + 1 not in fwd_done:
            fwd_done[blk + 1] = emit_fwd(blk + 1)
        for ft in range(NFT):
            uc, us = fwd[ft]
            uc3 = uc[:].rearrange("p (b h) -> p b h", b=4)
            us3 = us[:].rearrange("p (b h) -> p b h", b=4)

            # pointwise: Yr = Uc*A - Us*BC ; Yi = Uc*BC + Us*D
            dten = krD0 if ft == 0 else krA[ft]
            t1 = p_tmp.tile([128, 512], dt.float32, tag="t1")
            t2 = p_tmp.tile([128, 512], dt.float32, tag="t2")
            TT(t1[:].rearrange("p (b h) -> p b h", b=4), uc3, kb(krA[ft]), op.mult)
            GT(t2[:].rearrange("p (b h) -> p b h", b=4), us3, kb(krBC[ft]), op.mult)
            yr = p_yf.tile([128, 512], dt.float32r, tag="yf")
            TT(yr[:], t1[:], t2[:], op.subtract)
            t3 = p_tmp.tile([128, 512], dt.float32, tag="t1")
            t4 = p_tmp.tile([128, 512], dt.float32, tag="t2")
            GT(t3[:].rearrange("p (b h) -> p b h", b=4), uc3, kb(krBC[ft]), op.mult)
            TT(t4[:].rearrange("p (b h) -> p b h", b=4), us3, kb(dten), op.mult)
            yi = p_yf.tile([128, 512], dt.float32r, tag="yf")
            TT(yi[:], t3[:], t4[:], op.add)
            yr_t.append(yr)
            yi_t.append(yi)
        for lt in range(NLT):
            py = p_ps.tile([128, 512], dt.float32, tag="py", bufs=2)
            for ft in range(NFT):
                T.matmul(py[:], ai_tile(0, ft, lt), yr_t[ft][:],
                         start=(ft == 0), stop=False)
                T.matmul(py[:], ai_tile(1, ft, lt), yi_t[ft][:],
                         start=False, stop=(ft == NFT - 1))
            c_out = 4 * blk + lt
            yo = p_yout.tile([128, 512], dt.float32, tag="yout")
            A.copy(yo[:], py[:])
            eng = nc.sync if lt % 2 == 0 else nc.scalar
            eng.dma_start(y_ap[:, 128 * c_out:128 * c_out + 128, :].transpose([1, 0, 2]),
                          yo[:].rearrange("p (b h) -> p b h", b=4))


def _build_program():
    if _prog.built:
        return
    nc = bacc.Bacc("TRN2", target_bir_lowering=False, debug=False,
                   num_devices=NCORES)
    aps = {}
    aps["u"] = nc.dram_tensor("u", [B, L, HS], dt.float32r, kind="ExternalInput").ap()
    aps["D"] = nc.dram_tensor("D", [1, HS], dt.float32, kind="ExternalInput").ap()
    aps["logdt"] = nc.dram_tensor("logdt", [1, HS], dt.float32, kind="ExternalInput").ap()
    aps["Wre"] = nc.dram_tensor("Wre", [N, HS], dt.float32, kind="ExternalInput").ap()
    aps["Wim"] = nc.dram_tensor("Wim", [N, HS], dt.float32, kind="ExternalInput").ap()
    aps["Lre"] = nc.dram_tensor("Lre", [1, N], dt.float32, kind="ExternalInput").ap()
    aps["Lim"] = nc.dram_tensor("Lim", [1, N], dt.float32, kind="ExternalInput").ap()
    aps["CF"] = nc.dram_tensor("CF", [NJ, 2, NFT, 128, 128], dt.float32r,
                               kind="ExternalInput").ap()
    aps["AI"] = nc.dram_tensor("AI", [2, NFT, NLT, 128, 128], dt.float32r,
                               kind="ExternalInput").ap()
    aps["y"] = nc.dram_tensor("y", [B, L, HS], dt.float32, kind="ExternalOutput").ap()
    with tile.TileContext(nc, trace_sim=False) as tc:
        with ExitStack() as ctx:
            _emit_kernel(nc, tc, ctx, aps)
    nc.compile()
    _prog.nc = nc
    _prog.CF, _prog.AI = build_constants()
    _prog.built = True


def make_in_maps(u, D, log_dt, W_re, W_im, Lambda_re, Lambda_im):
    _build_program()
    in_maps = []
    for c in range(NCORES):
        h0 = c * HS
        in_maps.append({
            "u": np.ascontiguousarray(u[:, :, h0:h0 + HS], dtype=f32),
            "D": np.ascontiguousarray(D[h0:h0 + HS], dtype=f32).reshape(1, HS),
            "logdt": np.ascontiguousarray(log_dt[h0:h0 + HS], dtype=f32).reshape(1, HS),
            "Wre": np.ascontiguousarray(W_re[h0:h0 + HS].T, dtype=f32),
            "Wim": np.ascontiguousarray(W_im[h0:h0 + HS].T, dtype=f32),
            "Lre": np.ascontiguousarray(Lambda_re, dtype=f32).reshape(1, N),
            "Lim": np.ascontiguousarray(Lambda_im, dtype=f32).reshape(1, N),
            "CF": _prog.CF,
            "AI": _prog.AI,
        })
    return in_maps


LAST_RESULTS = None


def kernel(u, D, Lambda_re, Lambda_im, log_dt, W_re, W_im):
    global LAST_RESULTS
    from concourse.bass_utils import run_bass_kernel_spmd
    in_maps = make_in_maps(u, D, log_dt, W_re, W_im, Lambda_re, Lambda_im)
    res = run_bass_kernel_spmd(_prog.nc, in_maps, core_ids=list(range(NCORES)))
    LAST_RESULTS = res
    y = np.concatenate([res.results[c]["y"] for c in range(NCORES)], axis=2)
    return y.astype(np.float32)


# revision 56
# speedup vs baseline: 1.0117x; 1.0117x over previous
"""DSS (Diagonal State Space) layer as a Bass/Tile kernel for 8 Trainium2 NeuronCores.

Algorithm (per core, channels H sharded 8 x 128):
  1. Build the DSS-exp kernel k[l,h] = Re(sum_n W[h,n] z[h,n]^l), z = exp(dt_h * Lambda_n),
     on-device via a two-level power factorization l = 32a + b:
       GW[h,n,b] = W * z^b (b<32),  Z32[h,n,a] = z^(32a) (a<16), both by complex doubling,
     then a per-channel PE matmul contracts the 64 modes (re/im packed into 128 partitions).
  2. K_f = rfft_1024(k) via PE matmuls against host-precomputed DFT tiles.
  3. Overlap-save FFT convolution: per 512-sample block, forward rfft-1024 as PE matmuls
     (packed 512-frequency layout, Nyquist folded into the sin-tile f=0 slot), complex
     pointwise multiply split across DVE/GPSIMD, inverse rfft as PE matmuls producing the
     valid 512 samples.
  4. The skip connection y += u * D is folded into the frequency-domain filter
     (K'_f = K_f + D), so it costs nothing in the main loop.

All matmuls use float32r (full PE rate; ~1.6e-4 relative rounding). Transcendentals are
evaluated with small-argument polynomials on DVE (the ACT LUTs are only ~1e-4 accurate,
which would compound through the z^511 power chains). The forward runs one block ahead
of the inverse in the PE stream; DMA traffic is spread across the SP/ACT HWDGE queues.
"""

import sys

for _p in ("/opt/trn_rl_repo", "/opt/trn_rl_repo/concourse"):
    if _p not in sys.path:
        sys.path.insert(0, _p)

import numpy as np
from contextlib import ExitStack

import concourse.bacc as bacc
import concourse.tile as tile
import concourse.mybir as mybir

dt = mybir.dt
f32 = np.float32

B, L, H, N = 4, 4096, 1024, 64
LK = 512
F = 1024          # FFT length (overlap-save)
HOP = 512         # block hop
NCORES = 8
HS = H // NCORES  # 128 channels per core
NBLK = L // HOP   # 8
NFT = 4           # packed frequency tiles (512 freqs + Nyquist folded)
NJ = F // 128     # 8 contraction chunks for the forward DFT
NLT = HOP // 128  # 4 output l-tiles per block
NCH = L // 128    # 32 u chunks per core


# ---------------------------------------------------------------- host constants
def build_constants():
    l = np.arange(F, dtype=np.float64)[:, None]
    f = np.arange(512, dtype=np.float64)[None, :]
    ang = 2 * np.pi * l * f / F
    C = np.cos(ang)
    S = -np.sin(ang)
    S[:, 0] = (-1.0) ** np.arange(F)      # Nyquist row packed into sin-tile col 0
    CF = np.zeros((NJ, 2, NFT, 128, 128))
    for j in range(NJ):
        for ft in range(NFT):
            CF[j, 0, ft] = C[128 * j:128 * j + 128, 128 * ft:128 * ft + 128]
            CF[j, 1, ft] = S[128 * j:128 * j + 128, 128 * ft:128 * ft + 128]
    lc = 512 + np.arange(512, dtype=np.float64)[None, :]   # valid circular outputs
    fr = np.arange(512, dtype=np.float64)[:, None]
    cf_ = np.where(fr == 0, 1.0, 2.0)
    Ar = cf_ * np.cos(2 * np.pi * fr * lc / F) / F
    Ai = -(2.0 / F) * np.sin(2 * np.pi * fr * lc / F)
    Ai[0, :] = ((-1.0) ** lc[0]) / F                        # Nyquist inverse row
    AI = np.zeros((2, NFT, NLT, 128, 128))
    for ft in range(NFT):
        for lt in range(NLT):
            AI[0, ft, lt] = Ar[128 * ft:128 * ft + 128, 128 * lt:128 * lt + 128]
            AI[1, ft, lt] = Ai[128 * ft:128 * ft + 128, 128 * lt:128 * lt + 128]
    return CF.astype(f32), AI.astype(f32)


# Horner coefficient lists (highest degree first)
def _fact(k):
    r = 1.0
    for i in range(2, k + 1):
        r *= i
    return r


EXP10 = [1.0 / _fact(k) for k in range(10, -1, -1)]          # e^x, |x| <~ 0.9
EXP9 = [1.0 / _fact(k) for k in range(9, -1, -1)]            # e^x, |x| <~ 0.4
SIN9 = [1.0 / _fact(9), -1.0 / _fact(7), 1.0 / _fact(5), -1.0 / _fact(3), 1.0]   # odd, in u = x^2
COSC = [1.0 / _fact(10), -1.0 / _fact(8), 1.0 / _fact(6), -1.0 / _fact(4),
        1.0 / _fact(2)]          # cos(x) = 1 - u*POLY? see _cos_poly


class _Prog:
    def __init__(self):
        self.nc = None
        self.built = False


_prog = _Prog()


def _emit_kernel(nc, tc, ctx, aps):
    V = nc.vector
    A = nc.scalar
    T = nc.tensor
    u_ap = aps["u"]; y_ap = aps["y"]
    cf_ap = aps["CF"]; ai_ap = aps["AI"]
    TT = V.tensor_tensor
    GT = nc.gpsimd.tensor_tensor
    op = mybir.AluOpType

    # ---------------- pools
    p_cf = ctx.enter_context(tc.tile_pool(name="cf", bufs=1))
    p_ai = ctx.enter_context(tc.tile_pool(name="ai", bufs=1))
    p_uch = ctx.enter_context(tc.tile_pool(name="uch", bufs=8))
    p_yf = ctx.enter_context(tc.tile_pool(name="yf", bufs=8))
    p_krep = ctx.enter_context(tc.tile_pool(name="krep", bufs=1))
    p_tmp = ctx.enter_context(tc.tile_pool(name="tmp", bufs=2))
    p_yout = ctx.enter_context(tc.tile_pool(name="yout", bufs=2))
    p_kc = ctx.enter_context(tc.tile_pool(name="kc", bufs=4))
    p_gw = ctx.enter_context(tc.tile_pool(name="gw", bufs=1))
    p_z32 = ctx.enter_context(tc.tile_pool(name="z32", bufs=1))
    p_zp = ctx.enter_context(tc.tile_pool(name="zp", bufs=9))
    p_small = ctx.enter_context(tc.tile_pool(name="small", bufs=1))
    p_gwtmp = ctx.enter_context(tc.tile_pool(name="gwtmp", bufs=1))
    p_drep = ctx.enter_context(tc.tile_pool(name="drep", bufs=1))
    p_ps = ctx.enter_context(tc.tile_pool(name="ps", bufs=6, space="PSUM"))
    p_psk = ctx.enter_context(tc.tile_pool(name="psk", bufs=2, space="PSUM"))

    def fview(t):
        return t[:].bitcast(dt.float32)

    # ---------------- small parameter tiles first (they gate the whole k prologue)
    logdt = p_small.tile([1, HS], dt.float32, tag="logdt")
    A.dma_start(logdt[:], aps["logdt"][:])
    Lre = p_small.tile([1, N], dt.float32, tag="lre")
    A.dma_start(Lre[:], aps["Lre"][:])
    Lim_r = p_small.tile([1, N], dt.float32, tag="lim")
    A.dma_start(Lim_r[:], aps["Lim"][:])

    # ---------------- constant stationaries, merged DMAs spread over both HWDGE queues
    cf_big = {}
    _cfq = [0]

    def load_cf(j, t_):
        tl = p_cf.tile([128, 4, 128], dt.float32r, tag=f"cf{j}_{t_}", name=f"cfb{j}_{t_}")
        eng = (nc.sync, nc.scalar)[_cfq[0] % 2]
        _cfq[0] += 1
        eng.dma_start(tl[:], cf_ap[j, t_].transpose([1, 0, 2]))
        cf_big[(j, t_)] = tl

    for j in (4, 5, 6, 7):
        for t_ in range(2):
            load_cf(j, t_)

    def cf_tile(j, t_, ft):
        return cf_big[(j, t_)][:, ft, :]


    # u chunks: prefetch the first window up front so the in-order DMA queues
    # don't head-of-line block them behind the k-prologue traffic
    chunks = {}

    def get_chunk(c):
        assert c >= 0
        if c not in chunks:
            t_u = p_uch.tile([128, 4, 128], dt.float32r, tag="uch", name=f"uch{c}")
            eng = nc.sync if c % 2 == 0 else nc.scalar
            eng.dma_start(t_u[:], u_ap[:, 128 * c:128 * c + 128, :].transpose([1, 0, 2]))
            chunks[c] = t_u
        return chunks[c]

    for c in range(8):
        get_chunk(c)

    for j in (0, 1, 2, 3):
        for t_ in range(2):
            load_cf(j, t_)

    # inverse stationaries load after the forward-critical traffic
    ai_big = {}
    for t_ in range(2):
        for ft in range(NFT):
            tl = p_ai.tile([128, 4, 128], dt.float32r, tag=f"ai{t_}_{ft}", name=f"aib{t_}_{ft}")
            eng = nc.sync if (ft + t_) % 2 == 0 else nc.scalar
            eng.dma_start(tl[:], ai_ap[t_, ft].transpose([1, 0, 2]))
            ai_big[(t_, ft)] = tl

    def ai_tile(t_, ft, lt):
        return ai_big[(t_, ft)][:, lt, :]

    def emit_fwd(blk):
        out = []
        for ft in range(NFT):
            pc = p_ps.tile([128, 512], dt.float32, tag="ps", name=f"pc{blk}_{ft}")
            psn = p_ps.tile([128, 512], dt.float32, tag="ps", name=f"psn{blk}_{ft}")
            first = True
            for j in range(NJ):
                c = 4 * blk - 4 + j
                if c < 0:
                    continue
                ch = get_chunk(c)
                T.matmul(pc[:], cf_tile(j, 0, ft), ch[:].rearrange("p b h -> p (b h)"),
                         start=first, stop=(j == NJ - 1))
                T.matmul(psn[:], cf_tile(j, 1, ft), ch[:].rearrange("p b h -> p (b h)"),
                         start=first, stop=(j == NJ - 1))
                first = False
            # evacuate psum on ACT so the DVE/GPS pointwise runs all-SBUF
            uc = p_tmp.tile([128, 512], dt.float32, tag="uc", bufs=7, name=f"uc{blk}_{ft}")
            A.copy(uc[:], pc[:])
            us = p_tmp.tile([128, 512], dt.float32, tag="us", bufs=7, name=f"us{blk}_{ft}")
            A.copy(us[:], psn[:])
            out.append((uc, us))
        return out

    fwd_done = {0: emit_fwd(0), 1: emit_fwd(1)}
    PREFETCH3 = True

    def horner_exp(dst, x, coefs):
        # dst = e^x via Horner in x; dst/x are [p, w] fp32 APs; uses p_small temps
        p = dst
        V.memset(p, float(coefs[0]))
        for c in coefs[1:]:
            tq = p_small.tile([x.shape[0], x.shape[1]], dt.float32, tag="horner", bufs=2)
            TT(tq[:], p, x, op.mult)
            V.tensor_scalar_add(p, tq[:], float(c))

    # dt = exp(logdt) = (exp(logdt/8))^8
    x8 = p_small.tile([1, HS], dt.float32, tag="x8")
    V.tensor_scalar_mul(x8[:], logdt[:], 0.125)
    e8 = p_small.tile([1, HS], dt.float32, tag="e8")
    horner_exp(e8[:], x8[:], EXP10)
    dtv = p_small.tile([1, HS], dt.float32, tag="dtv")
    t_a = p_small.tile([1, HS], dt.float32, tag="sq1")
    TT(t_a[:], e8[:], e8[:], op.mult)
    t_b = p_small.tile([1, HS], dt.float32, tag="sq2")
    TT(t_b[:], t_a[:], t_a[:], op.mult)
    TT(dtv[:], t_b[:], t_b[:], op.mult)

    # -exp(Lre) = -(exp(Lre/8))^8
    xl = p_small.tile([1, N], dt.float32, tag="xl")
    V.tensor_scalar_mul(xl[:], Lre[:], 0.125)
    el8 = p_small.tile([1, N], dt.float32, tag="el8")
    horner_exp(el8[:], xl[:], EXP10)
    t_c = p_small.tile([1, N], dt.float32, tag="sq3")
    TT(t_c[:], el8[:], el8[:], op.mult)
    t_d = p_small.tile([1, N], dt.float32, tag="sq4")
    TT(t_d[:], t_c[:], t_c[:], op.mult)
    negel = p_small.tile([1, N], dt.float32, tag="negel")
    t_e = p_small.tile([1, N], dt.float32, tag="sq5")
    TT(t_e[:], t_d[:], t_d[:], op.mult)
    V.tensor_scalar_mul(negel[:], t_e[:], -1.0)

    # outer products: a[n,h] = -e^{Lre_n} dt_h ; b[n,h] = Lim_n dt_h
    ps_a = p_psk.tile([128, 512], dt.float32, tag="psk")
    T.matmul(ps_a[0:N, 0:HS], negel[:], dtv[:], start=True, stop=True)
    ps_b = p_psk.tile([128, 512], dt.float32, tag="psk")
    T.matmul(ps_b[0:N, 0:HS], Lim_r[:], dtv[:], start=True, stop=True)

    # half-angle pieces on [N, HS]
    ah = p_small.tile([N, HS], dt.float32, tag="ah")
    V.tensor_scalar_mul(ah[:], ps_a[0:N, 0:HS], 0.5)
    bh = p_small.tile([N, HS], dt.float32, tag="bh")
    V.tensor_scalar_mul(bh[:], ps_b[0:N, 0:HS], 0.5)
    ea = p_small.tile([N, HS], dt.float32, tag="ea")
    horner_exp(ea[:], ah[:], EXP9)
    # sin(bh), cos(bh) via u = bh^2
    ub = p_small.tile([N, HS], dt.float32, tag="ub")
    TT(ub[:], bh[:], bh[:], op.mult)
    sp = p_small.tile([N, HS], dt.float32, tag="sp")
    V.memset(sp[:], float(SIN9[0]))
    for c in SIN9[1:]:
        tq = p_small.tile([N, HS], dt.float32, tag="horner", bufs=2)
        TT(tq[:], sp[:], ub[:], op.mult)
        V.tensor_scalar_add(sp[:], tq[:], float(c))
    sb = p_small.tile([N, HS], dt.float32, tag="sb")
    TT(sb[:], sp[:], bh[:], op.mult)          # sin(b/2)
    cp = p_small.tile([N, HS], dt.float32, tag="cp")
    V.memset(cp[:], float(COSC[0]))
    for c in COSC[1:]:
        tq = p_small.tile([N, HS], dt.float32, tag="horner", bufs=2)
        TT(tq[:], cp[:], ub[:], op.mult)
        V.tensor_scalar_add(cp[:], tq[:], float(c))
    # cos(x) = 1 - u * cp  (cp = 1/2 - u/24 + ... evaluated via Horner above)
    cb = p_small.tile([N, HS], dt.float32, tag="cb")
    tq = p_small.tile([N, HS], dt.float32, tag="horner", bufs=2)
    TT(tq[:], cp[:], ub[:], op.mult)
    V.tensor_scalar(cb[:], tq[:], -1.0, 1.0, op.mult, op.add)

    wre = p_small.tile([N, HS], dt.float32, tag="wre")
    TT(wre[:], ea[:], cb[:], op.mult)
    wim = p_small.tile([N, HS], dt.float32, tag="wim")
    TT(wim[:], ea[:], sb[:], op.mult)

    # complex squaring on separate re/im planes (all base-partition 0, lane-aligned)
    def csq_parts(dre, dim_, sre, sim):
        t1 = p_small.tile([N, HS], dt.float32, tag="csq1", bufs=2)
        TT(t1[:], sre, sre, op.mult)
        t2 = p_small.tile([N, HS], dt.float32, tag="csq2", bufs=2)
        TT(t2[:], sim, sim, op.mult)
        TT(dre, t1[:], t2[:], op.subtract)
        t3 = p_small.tile([N, HS], dt.float32, tag="csq3", bufs=2)
        TT(t3[:], sre, sim, op.mult)
        V.tensor_scalar_mul(dim_, t3[:], 2.0)

    def new_zpair(nm):
        zr = p_zp.tile([N, HS], dt.float32, tag="zp", name=f"{nm}r")
        zi = p_zp.tile([N, HS], dt.float32, tag="zp", name=f"{nm}i")
        return zr, zi


    # ---------------- GW planes [N, HS, 32] holding (Re, -Im) of W z^b
    GWre_r = p_gw.tile([N, HS, 32], dt.float32r, tag="gwre")
    GWim_r = p_gw.tile([N, HS, 32], dt.float32r, tag="gwim")   # stores -Im
    GWre = GWre_r[:]
    GWim = GWim_r[:]
    A.dma_start(GWre[:, :, 0], aps["Wre"][:].bitcast(dt.float32r))
    wimt = p_small.tile([N, HS], dt.float32, tag="wimt")
    A.dma_start(wimt[:], aps["Wim"][:])
    V.tensor_scalar_mul(GWim[:, :, 0], wimt[:], -1.0)

    def cdouble_seg(pre, pim, zr, zi, s0, d0, w, conj_stored):
        # planes [.., d0:d0+w] = planes[.., s0:s0+w] * (zr + i zi);
        # when conj_stored, the im plane holds the negated imaginary part.
        zre = zr[:].unsqueeze(2).broadcast_to([N, HS, w])
        zim = zi[:].unsqueeze(2).broadcast_to([N, HS, w])
        t2 = p_gwtmp.tile([N, HS, 8], dt.float32, tag="gt2", bufs=3)
        t4 = p_gwtmp.tile([N, HS, 8], dt.float32, tag="gt2", bufs=3)
        TT(pre[:, :, d0:d0 + w], pre[:, :, s0:s0 + w], zre, op.mult)
        GT(t2[:, :, 0:w], pim[:, :, s0:s0 + w], zim, op.mult)
        TT(pim[:, :, d0:d0 + w], pim[:, :, s0:s0 + w], zre, op.mult)
        GT(t4[:, :, 0:w], pre[:, :, s0:s0 + w], zim, op.mult)
        TT(pre[:, :, d0:d0 + w], pre[:, :, d0:d0 + w], t2[:, :, 0:w],
           op.add if conj_stored else op.subtract)
        TT(pim[:, :, d0:d0 + w], pim[:, :, d0:d0 + w], t4[:, :, 0:w],
           op.subtract if conj_stored else op.add)

    def cdouble(pre, pim, zr, zi, w, conj_stored):
        cdouble_seg(pre, pim, zr, zi, 0, w, w, conj_stored)


    # ---------------- Z32 planes [N, HS, 16] natural complex z^(32a)
    Zre_r = p_z32.tile([N, HS, 16], dt.float32r, tag="z32re")
    Zim_r = p_z32.tile([N, HS, 16], dt.float32r, tag="z32im")
    Zre = Zre_r[:]
    Zim = Zim_r[:]
    # a=0 plane is the complex constant 1+0i (memset can't emit float32r)
    V.tensor_scalar(Zre[:, :, 0], wre[:], 0.0, 1.0, op.mult, op.add)
    V.tensor_scalar(Zim[:, :, 0], wre[:], 0.0, 0.0, op.mult, op.add)

    # interleaved power chain + doubling: GW level j follows zp[j] immediately,
    # Z32 level j follows za[j], keeping the serial latency to a minimum
    zp = []
    z0 = new_zpair("z0")
    csq_parts(z0[0][:], z0[1][:], wre[:], wim[:])
    zp.append(z0)
    cdouble(GWre, GWim, zp[0][0], zp[0][1], 1, conj_stored=True)
    for j in range(1, 5):                     # z^2, z^4, z^8, z^16
        zj = new_zpair(f"z{1 << j}")
        csq_parts(zj[0][:], zj[1][:], zp[-1][0][:], zp[-1][1][:])
        zp.append(zj)
        if j < 4:
            cdouble(GWre, GWim, zp[j][0], zp[j][1], 1 << j, conj_stored=True)
    za = []
    z32t = new_zpair("z32")
    csq_parts(z32t[0][:], z32t[1][:], zp[4][0][:], zp[4][1][:])
    za.append(z32t)                           # z^32
    cdouble_seg(GWre, GWim, zp[4][0], zp[4][1], 0, 16, 8, conj_stored=True)
    cdouble_seg(GWre, GWim, zp[4][0], zp[4][1], 8, 24, 8, conj_stored=True)
    cdouble(Zre, Zim, za[0][0], za[0][1], 1, conj_stored=False)
    for j in range(1, 4):                     # z^64, z^128, z^256
        zj = new_zpair(f"za{j}")
        csq_parts(zj[0][:], zj[1][:], za[-1][0][:], za[-1][1][:])
        za.append(zj)
        cdouble(Zre, Zim, za[j][0], za[j][1], 1 << j, conj_stored=False)

    # ---------------- mode-sum: k[32a+b, h], two contraction-64 matmuls per channel
    # psum += GWre_h^T @ Zre_h ; psum += GWim_h^T @ Zim_h  (im plane is negated)
    ks = []
    for g in range(4):
        kp_g = p_psk.tile([32, 32, 16], dt.float32, tag="psk", name=f"kp{g}")
        for hl in range(32):
            h = 32 * g + hl
            T.matmul(kp_g[0:32, hl, :], GWre_r[:, h, :], Zre_r[:, h, :],
                     start=True, stop=False)
            T.matmul(kp_g[0:32, hl, :], GWim_r[:, h, :], Zim_r[:, h, :],
                     start=False, stop=True)
        # evacuate lane-aligned with a-major free order (contiguous shuffle reads)
        t_ks = p_yout.tile([32, 16, 32], dt.float32r, tag="yout", name=f"ks{g}")
        A.copy(t_ks[:], kp_g[:].transpose([0, 2, 1]))
        ks.append(t_ks)
    kc = []
    for c in range(4):
        kc.append(p_kc.tile([128, 128], dt.float32r, tag="kc", name=f"kc{c}"))
    kqi = 0
    for g in range(4):
        for c in range(4):
            for al in range(4):
                eng = (nc.sync, nc.scalar, nc.gpsimd)[kqi % 3]
                kqi += 1
                eng.dma_start(kc[c][:][32 * al:32 * al + 32, 32 * g:32 * g + 32],
                              ks[g][0:32, 4 * c + al, :])

    # ---------------- D_rep [128, 128] (D broadcast down partitions; folded into K)
    dtile = p_small.tile([1, HS], dt.float32, tag="dtile")
    A.dma_start(dtile[:], aps["D"][:])
    ones = p_small.tile([1, 128], dt.float32, tag="ones")
    V.memset(ones[:], 1.0)
    ps_d = p_psk.tile([128, 512], dt.float32, tag="psk")
    T.matmul(ps_d[0:128, 0:HS], ones[:], dtile[:], start=True, stop=True)
    D_rep = p_drep.tile([128, 128], dt.float32, tag="drep")
    A.copy(D_rep[:], ps_d[0:128, 0:HS])

    # ---------------- K_f via packed DFT (reuse forward stationaries j=0..3)
    kdft_ps = {}
    pks = {}
    for t_ in range(2):
        pks[t_] = p_psk.tile([128, 4, 128], dt.float32, tag="psk", name=f"kdft{t_}")
    for ft in range(NFT):
        for t_ in range(2):
            for c in range(4):
                T.matmul(pks[t_][:, ft, :], cf_tile(c, t_, ft), kc[c][:],
                         start=(c == 0), stop=(c == 3))
            kdft_ps[(t_, ft)] = pks[t_][:, ft, :]

    # Krep tensors [128, 128]; the pointwise broadcasts them across the 4 batch groups
    zrow = p_small.tile([1, 128], dt.float32, tag="zrow")
    V.memset(zrow[:], 0.0)
    # the skip connection u*D folds into the filter: K'_f = K_f + D (real part, all f)
    krA, krBC = [], []
    for ft in range(NFT):
        ta = p_krep.tile([128, 128], dt.float32r, tag=f"krA{ft}")
        tb = p_krep.tile([128, 128], dt.float32r, tag=f"krB{ft}")
        TT(ta[:], kdft_ps[(0, ft)], D_rep[:], op.add)
        A.copy(tb[:], kdft_ps[(1, ft)])
        krA.append(ta)
        krBC.append(tb)
    krD0 = p_krep.tile([128, 128], dt.float32r, tag="krD0")
    TT(krD0[:], kdft_ps[(0, 0)], D_rep[:], op.add)
    # row 0 of D-tensor holds K512r (from the packed sin psum row 0), plus D
    TT(krD0[0:1, :], kdft_ps[(1, 0)][0:1, :], D_rep[0:1, :], op.add)
    V.tensor_scalar(krBC[0][0:1, :], zrow[:], 0.0, 0.0, op.mult, op.add)     # Ki slot for f=0/Nyquist is zero

    # ---------------- D_rep [128, 512]
    dtile = p_small.tile([1, HS], dt.float32, tag="dtile")
    A.dma_start(dtile[:], aps["D"][:])
    ones = p_small.tile([1, 128], dt.float32, tag="ones")
    V.memset(ones[:], 1.0)
    ps_d = p_psk.tile([128, 512], dt.float32, tag="psk")
    T.matmul(ps_d[0:128, 0:HS], ones[:], dtile[:], start=True, stop=True)
    D_rep = p_drep.tile([128, 128], dt.float32, tag="drep")
    A.copy(D_rep[:], ps_d[0:128, 0:HS])

    # ---------------- main loop: overlap-save blocks

    def kb(t):
        return t[:].unsqueeze(1).broadcast_to([128, 4, 128])

    fwd_done[2] = emit_fwd(2)

    for blk in range(NBLK):
        yr_t, yi_t = [], []
        fwd = fwd_done.pop(blk)
        if blk + 1 < NBLK and blk + 1 not in fwd_done:
            fwd_done[blk + 1] = emit_fwd(blk + 1)
        for ft in range(NFT):
            uc, us = fwd[ft]
            uc3 = uc[:].rearrange("p (b h) -> p b h", b=4)
            us3 = us[:].rearrange("p (b h) -> p b h", b=4)

            # pointwise: Yr = Uc*A - Us*BC ; Yi = Uc*BC + Us*D
            dten = krD0 if ft == 0 else krA[ft]
            t1 = p_tmp.tile([128, 512], dt.float32, tag="t1")
            t2 = p_tmp.tile([128, 512], dt.float32, tag="t2")
            TT(t1[:].rearrange("p (b h) -> p b h", b=4), uc3, kb(krA[ft]), op.mult)
            GT(t2[:].rearrange("p (b h) -> p b h", b=4), us3, kb(krBC[ft]), op.mult)
            yr = p_yf.tile([128, 512], dt.float32r, tag="yf")
            TT(yr[:], t1[:], t2[:], op.subtract)
            t3 = p_tmp.tile([128, 512], dt.float32, tag="t1")
            t4 = p_tmp.tile([128, 512], dt.float32, tag="t2")
            GT(t3[:].rearrange("p (b h) -> p b h", b=4), uc3, kb(krBC[ft]), op.mult)
            TT(t4[:].rearrange("p (b h) -> p b h", b=4), us3, kb(dten), op.mult)
            yi = p_yf.tile([128, 512], dt.float32r, tag="yf")
            TT(yi[:], t3[:], t4[:], op.add)
            yr_t.append(yr)
            yi_t.append(yi)
        for lt in range(NLT):
            py = p_ps.tile([128, 512], dt.float32, tag="ps")
            for ft in range(NFT):
                T.matmul(py[:], ai_tile(0, ft, lt), yr_t[ft][:],
                         start=(ft == 0), stop=False)
                T.matmul(py[:], ai_tile(1, ft, lt), yi_t[ft][:],
                         start=False, stop=(ft == NFT - 1))
            c_out = 4 * blk + lt
            yo = p_yout.tile([128, 512], dt.float32, tag="yout")
            A.copy(yo[:], py[:])
            eng = nc.sync if lt % 2 == 0 else nc.scalar
            eng.dma_start(y_ap[:, 128 * c_out:128 * c_out + 128, :].transpose([1, 0, 2]),
                          yo[:].rearrange("p (b h) -> p b h", b=4))


def _build_program():
    if _prog.built:
        return
    nc = bacc.Bacc("TRN2", target_bir_lowering=False, debug=False,
                   num_devices=NCORES)
    aps = {}
    aps["u"] = nc.dram_tensor("u", [B, L, HS], dt.float32r, kind="ExternalInput").ap()
    aps["D"] = nc.dram_tensor("D", [1, HS], dt.float32, kind="ExternalInput").ap()
    aps["logdt"] = nc.dram_tensor("logdt", [1, HS], dt.float32, kind="ExternalInput").ap()
    aps["Wre"] = nc.dram_tensor("Wre", [N, HS], dt.float32, kind="ExternalInput").ap()
    aps["Wim"] = nc.dram_tensor("Wim", [N, HS], dt.float32, kind="ExternalInput").ap()
    aps["Lre"] = nc.dram_tensor("Lre", [1, N], dt.float32, kind="ExternalInput").ap()
    aps["Lim"] = nc.dram_tensor("Lim", [1, N], dt.float32, kind="ExternalInput").ap()
    aps["CF"] = nc.dram_tensor("CF", [NJ, 2, NFT, 128, 128], dt.float32r,
                               kind="ExternalInput").ap()
    aps["AI"] = nc.dram_tensor("AI", [2, NFT, NLT, 128, 128], dt.float32r,
                               kind="ExternalInput").ap()
    aps["y"] = nc.dram_tensor("y", [B, L, HS], dt.float32, kind="ExternalOutput").ap()
    with tile.TileContext(nc, trace_sim=False) as tc:
        with ExitStack() as ctx:
            _emit_kernel(nc, tc, ctx, aps)
    nc.compile()
    _prog.nc = nc
    _prog.CF, _prog.AI = build_constants()
    _prog.built = True


def make_in_maps(u, D, log_dt, W_re, W_im, Lambda_re, Lambda_im):
    _build_program()
    in_maps = []
    for c in range(NCORES):
        h0 = c * HS
        in_maps.append({
            "u": np.ascontiguousarray(u[:, :, h0:h0 + HS], dtype=f32),
            "D": np.ascontiguousarray(D[h0:h0 + HS], dtype=f32).reshape(1, HS),
            "logdt": np.ascontiguousarray(log_dt[h0:h0 + HS], dtype=f32).reshape(1, HS),
            "Wre": np.ascontiguousarray(W_re[h0:h0 + HS].T, dtype=f32),
            "Wim": np.ascontiguousarray(W_im[h0:h0 + HS].T, dtype=f32),
            "Lre": np.ascontiguousarray(Lambda_re, dtype=f32).reshape(1, N),
            "Lim": np.ascontiguousarray(Lambda_im, dtype=f32).reshape(1, N),
            "CF": _prog.CF,
            "AI": _prog.AI,
        })
    return in_maps


LAST_RESULTS = None


def kernel(u, D, Lambda_re, Lambda_im, log_dt, W_re, W_im):
    global LAST_RESULTS
    from concourse.bass_utils import run_bass_kernel_spmd
    in_maps = make_in_maps(u, D, log_dt, W_re, W_im, Lambda_re, Lambda_im)
    res = run_bass_kernel_spmd(_prog.nc, in_maps, core_ids=list(range(NCORES)))
    LAST_RESULTS = res
    y = np.concatenate([res.results[c]["y"] for c in range(NCORES)], axis=2)
    return y.astype(np.float32)
